# revision 28
# baseline (speedup 1.0000x reference)
"""Multi-head attention (B=4, S=2048, D=1024, H=16, HD=64) on 8 TRN2 NeuronCores.

Sharding: core c handles batch b = c//2 and head-group hg = c%2 (8 heads each).
Embarrassingly parallel over (b, head-group); QKV projection column-sharded.

v2 dataflow (all on-chip data fp16; PSUM f32):
  - Host passes X^T [D,S], W m-tiles, all bf16 (halves DMA + SBUF vs f32).
  - Projection: Q^T/K^T m-tiles [128, S] = W^T X via full-depth PSUM groups,
    DVE bias-add moves PSUM -> sbt (bf16). V per head: [128 seq, 64] chunks
    + a ones column -> V' [128, 8, 65].
  - Attention in 16 passes p = (head h, q-half qh). Per kpos-chunk kc:
      S^T[kc, qhalf] = K^T.T @ Q^T   (PE, 2x512 free)
      st = exp(S^T/8)                 (ScalarE activation, or GPSIMD pow with
                                       base e^(1/8) via a DVE PSUM->SBUF copy;
                                       split keeps ScalarE off the critical path)
  - AV runs one pass behind (st fully available): out[q, d] orientation —
    stationary st [128,128] slices, moving V' [128, 65]: 65-row streams cost
    half of the q-moving orientation on the PE (cost = moving rows only).
    Per q-chunk: 16-matmul PSUM group in one bank; softmax sums ride the
    ones column; DVE reciprocal + per-partition tensor_scalar_mul normalize
    (no partition broadcast needed).
  - Output staged bf16 [128, 16, 64] per head, one DMA per head; host
    transposes/concatenates and upcasts to f32.

The projection is woven into the pass schedule so the PE never idles:
m-tile pairs (g, 4+g) land before head-group g's first pass; V'(h) lands
before head h's first AV pass. exp chunks alternate ScalarE/GPSIMD.
"""

import numpy as np

import concourse.bass as bass
import concourse.mybir as mybir
import concourse.tile as tile
from concourse import bacc
from concourse.bass_utils import run_bass_kernel_spmd

F32 = mybir.dt.float32
FP16 = mybir.dt.float16
AF = mybir.ActivationFunctionType
ALU = mybir.AluOpType

P = 128          # partitions
D = 1024         # model dim
S = 2048         # sequence
HD = 64          # head dim
NHC = 8          # heads per core
QKC = NHC * HD   # 512 cols per core for each of Q, K, V
KD = D // P      # 8 contraction chunks
MS = S // P      # 16 kpos chunks
QH = 1024        # q-half width
NPASS = 16       # (head, q-half) passes
SCALE = 1.0 / 8.0
EXP_BASE = float(np.exp(SCALE))

N_CORES = 8
B_FULL = 4

# kc steps whose exp chunk runs on GPSIMD (pow) instead of ScalarE
POOL_KC = (1, 3, 6, 9, 12, 14)


def _build(iters=1):
    nc = bacc.Bacc(None, target_bir_lowering=False)

    xt = nc.dram_tensor("xt", [D, S], FP16, kind="ExternalInput")
    # wqk host-permuted: row (m*128 + p), col (k*128 + j) holds
    # W_qk[k*128 + p, m*128 + j] — one m-tile = contiguous [128, 1024] block
    wqk = nc.dram_tensor("wqk", [D, D], FP16, kind="ExternalInput")
    wv = nc.dram_tensor("wv", [D, QKC], FP16, kind="ExternalInput")
    bqk = nc.dram_tensor("bqk", [P, KD], F32, kind="ExternalInput")   # [p, m]
    bvb = nc.dram_tensor("bvb", [P, QKC], F32, kind="ExternalInput")  # replicated
    outd = nc.dram_tensor("outd", [NHC * P, MS * HD], FP16, kind="ExternalOutput")

    with tile.TileContext(nc) as tc:
        with (
            tc.tile_pool(name="persist", bufs=1) as pp,
            tc.tile_pool(name="psc", bufs=5, space="PSUM") as psc,
            tc.tile_pool(name="psx", bufs=3, space="PSUM") as psx,
        ):
            for it in range(iters):
                bqk_sb = pp.tile([P, KD], F32, tag="bqk", name=f"bqk{it}")
                bvb_sb = pp.tile([P, QKC], F32, tag="bvb", name=f"bvb{it}")
                expbase = pp.tile([P, QH], F32, tag="eb", name=f"eb{it}")
                # warmup matmul: starts the PE p-state ramp clock at t~0 so
                # the real prologue matmuls run at full clock
                if it == 0:
                    wmu = pp.tile([P, 8], FP16, tag="wmu", name="wmu")
                    nc.vector.memset(wmu[:], 0.5)
                    pwu = psx.tile([P, 512], F32, tag="ps1", name="pwu")
                    nc.tensor.matmul(pwu[0:8, 0:8], wmu[:], wmu[:],
                                     start=True, stop=True)
                nc.vector.memset(expbase[:], EXP_BASE)

                # --- input DMA on two HWDGE queues (sync + scalar) ---
                xt_sb = [pp.tile([P, S], FP16, tag=f"xt{k}", name=f"xt{it}_{k}")
                         for k in range(KD)]
                w_sb = [pp.tile([P, KD, P], FP16, tag=f"wm{m}", name=f"wm{it}_{m}")
                        for m in range(KD)]
                wv_sb = [pp.tile([P, QKC], FP16, tag=f"wv{k}", name=f"wv{it}_{k}")
                        for k in range(KD)]

                def dma_w(m, eng, it=it):
                    eng.dma_start(
                        out=w_sb[m][:],
                        in_=wqk[m * P:(m + 1) * P, :].rearrange(
                            "p (k j) -> p k j", k=KD))

                # two HWDGE queues; xt lands in column halves so full-depth
                # projection quanta on the first q-half start at ~5us
                def dma_xt(k, half, eng):
                    eng.dma_start(
                        out=xt_sb[k][:, half * QH:(half + 1) * QH],
                        in_=xt[k * P:(k + 1) * P, half * QH:(half + 1) * QH])

                dma_w(0, nc.sync)
                dma_w(4, nc.scalar)
                for k in (0, 2, 1, 3):
                    dma_xt(k, 0, nc.sync if k % 2 == 0 else nc.scalar)
                nc.sync.dma_start(out=bqk_sb[:], in_=bqk[:])
                for k in (4, 6, 5, 7):
                    dma_xt(k, 0, nc.sync if k % 2 == 0 else nc.scalar)
                nc.scalar.dma_start(out=bvb_sb[:], in_=bvb[:])
                for k in (0, 2):
                    nc.sync.dma_start(out=wv_sb[k][:],
                                      in_=wv[k * P:(k + 1) * P, :])
                for k in (1, 3):
                    nc.scalar.dma_start(out=wv_sb[k][:],
                                        in_=wv[k * P:(k + 1) * P, :])
                for k in (4, 5, 6, 7):
                    nc.gpsimd.dma_start(out=wv_sb[k][:],
                                        in_=wv[k * P:(k + 1) * P, :])
                for k in range(0, KD, 2):
                    dma_xt(k, 1, nc.sync)
                for k in range(1, KD, 2):
                    dma_xt(k, 1, nc.gpsimd)
                for m in (1, 5, 2, 3, 7):
                    dma_w(m, nc.scalar)
                dma_w(6, nc.sync)

                # persistent attention tensors
                sbt = [pp.tile([P, S], FP16, tag=f"sbt{m}", name=f"sbt{it}_{m}")
                       for m in range(KD)]
                vv = [pp.tile([P, NHC, HD + 1], FP16, tag=f"vv{k}",
                              name=f"vv{it}_{k}") for k in range(MS)]
                out_sb = [pp.tile([P, MS * HD], FP16, tag=f"ou{h}",
                                  name=f"ou{it}_{h}") for h in range(NHC)]

                # ---------- projection quanta ----------
                def m_quantum(m, qr, it=it):
                    """Full-depth [128,512] quarter of Q/K m-tile m."""
                    ps = psx.tile([P, 512], F32, tag="ps1",
                                  name=f"pm{it}_{m}_{qr}")
                    dst = ps[:]
                    for k in range(KD):
                        nc.tensor.matmul(dst, w_sb[m][:, k, :],
                                         xt_sb[k][:, qr * 512:(qr + 1) * 512],
                                         start=(k == 0), stop=(k == KD - 1))
                    nc.vector.tensor_scalar_add(
                        sbt[m][:, qr * 512:(qr + 1) * 512], dst,
                        bqk_sb[:, m:m + 1])

                def v_quantum(h, kc, it=it):
                    """V' chunk kc for head h: [128 seq, 64] + bias."""
                    ps = psx.tile([P, 512], F32, tag="ps1",
                                  name=f"pv{it}_{h}_{kc}")
                    dst = ps[:, 0:HD]
                    for k in range(KD):
                        nc.tensor.matmul(dst, xt_sb[k][:, kc * P:(kc + 1) * P],
                                         wv_sb[k][:, h * HD:(h + 1) * HD],
                                         start=(k == 0), stop=(k == KD - 1))
                    nc.vector.tensor_tensor(
                        out=vv[kc][:, h, 0:HD], in0=dst,
                        in1=bvb_sb[:, h * HD:(h + 1) * HD], op=ALU.add)
                    if h == 0:
                        nc.vector.memset(vv[kc][:, :, HD:HD + 1], 1.0)

                # ---------- deadline-driven weave schedule ----------
                # Each projection quantum has a deadline (pass, step) before
                # which it must be emitted. EDF + load-balancing fills every
                # pass to ~equal PE cycles so no pass goes exp-bound.
                quanta = []  # (cycles, deadline, earliest, spec)

                def _dl(p, s):
                    # emission at (p, s) happens after QK/AV of step s, so a
                    # consumer at (p, 0) means "end of pass p-1"
                    return (p - 1, MS) if s == 0 else (p, s)

                # earliest emission step in pass 0, tracking DMA landing of
                # the quantum's inputs (w m-tiles trail on the scalar queue,
                # xt column-half 1 and wv land mid-prologue)
                W_ES = {0: 0, 4: 0, 1: 5, 5: 6, 2: 7, 6: 8, 3: 9, 7: 10}
                for g in range(4):
                    for qr in range(4):
                        h1_es = 6 if qr >= 2 else 0
                        # Q-tile m=g: halves due at head-group g's q-half pass
                        quanta.append((4096, _dl(4 * g + (qr // 2), 0),
                                       (0, max(W_ES[g], h1_es)), ("m", g, qr)))
                        # K-tile m=4+g: quarters consumed incrementally (kpos)
                        quanta.append((4096, _dl(4 * g, 4 * qr),
                                       (0, max(W_ES[4 + g], h1_es)),
                                       ("m", 4 + g, qr)))
                for h in range(NHC):
                    for kc in range(MS):
                        if h == 0 and kc < 8:
                            continue  # emitted in the prologue
                        # V'(h) read by AV(2h) during pass 2h+1
                        quanta.append((512, (2 * h, MS),
                                       (0, 2 if kc < 8 else 6), ("v", h, kc)))

                quanta.sort(key=lambda q: q[1])
                prolog_q = [q for q in quanta if q[1][0] < 0]
                pending = [q for q in quanta if q[1][0] >= 0]

                def emit_quantum(spec):
                    kind, a, b = spec
                    if kind == "m":
                        m_quantum(a, b)
                    else:
                        v_quantum(a, b)

                CAP = 35000
                sched = {}
                pass_loads = []
                for p in range(NPASS):
                    load = 16384 + (8320 if p >= 1 else 0)
                    placed = []
                    rest = []
                    for cyc, (dp, ds), (ep, es), spec in pending:
                        ok = ep <= p or dp == p
                        if ok and (dp == p or (dp > p and load < CAP)):
                            placed.append((cyc, ds if dp == p else MS,
                                           es if ep == p else 0, spec))
                            load += cyc
                        else:
                            rest.append((cyc, (dp, ds), (ep, es), spec))
                    pending = rest
                    pass_loads.append(load)
                    # assign to steps: earliest step-deadline first, least
                    # loaded allowed step in [es, ds)
                    placed.sort(key=lambda q: q[1])
                    step_load = [0] * 8 + [1040] * 8 if p >= 1 else [0] * MS
                    for cyc, ds, es, spec in placed:
                        hi = max(es + 1, min(ds, MS))
                        s = min(range(es, hi), key=lambda i: step_load[i])
                        step_load[s] += cyc
                        sched.setdefault((p, s), []).append(spec)
                assert not pending, f"unscheduled quanta: {len(pending)}"
                import os
                if os.environ.get("KPRINT"):
                    print("pass loads:", pass_loads)

                # ---------- prologue ----------
                # The pre-pass-0 m-quanta run in two half-depth stages (k 0-3
                # after the first xt chunks land, k 4-7 later); V'(h0) chunks
                # 0-7 (xt half-0 only) fill the remaining DMA-landing stalls.
                prolog_m = [(a, b) for _, _, _, (k, a, b) in prolog_q]
                assert all(k == "m" for _, _, _, (k, _, _) in prolog_q)
                with tc.tile_pool(name=f"prolog{it}", bufs=1) as plp:
                    scrm = {
                        (m, qr): plp.tile([P, 512], F32, tag=f"scrm{m}_{qr}",
                                          name=f"scrm{it}_{m}_{qr}")
                        for m, qr in prolog_m
                    }

                    def m_stage(m, qr, k0, it=it):
                        ps = psx.tile([P, 512], F32, tag="ps1",
                                      name=f"pp{it}_{m}_{qr}_{k0}")
                        for j, k in enumerate(range(k0, k0 + 4)):
                            nc.tensor.matmul(
                                ps[:], w_sb[m][:, k, :],
                                xt_sb[k][:, qr * 512:(qr + 1) * 512],
                                start=(j == 0), stop=(j == 3))
                        if k0 == 0:
                            nc.vector.tensor_scalar_add(scrm[(m, qr)][:],
                                                        ps[:],
                                                        bqk_sb[:, m:m + 1])
                        else:
                            nc.vector.tensor_tensor(
                                out=sbt[m][:, qr * 512:(qr + 1) * 512],
                                in0=ps[:], in1=scrm[(m, qr)][:], op=ALU.add)

                    for m, qr in prolog_m:
                        m_stage(m, qr, 0)
                    for m, qr in prolog_m:
                        m_stage(m, qr, 4)
                    for kc in range(8):
                        v_quantum(0, kc)

                # ---------- attention passes ----------
                stp = tc.alloc_tile_pool(name=f"stp{it}", bufs=1)
                st_tiles = [[None] * MS, [None] * MS]
                av_cur = [None] * (QH // P)
                recp = stp

                def emit_qk(p, kc, it=it):
                    h, qh = p // 2, p % 2
                    g, off = h // 2, (h % 2) * HD
                    qt, kt = sbt[g], sbt[4 + g]
                    st = stp.tile([P, QH], FP16, tag="st", bufs=32,
                                  name=f"st{it}_{p}_{kc}")
                    st_tiles[p % 2][kc] = st
                    for j in range(2):
                        sc = psc.tile([P, 512], F32, tag="sc",
                                      name=f"sc{it}_{p}_{kc}_{j}")
                        nc.tensor.matmul(
                            sc[:],
                            kt[off:off + HD, kc * P:(kc + 1) * P],
                            qt[off:off + HD, qh * QH + j * 512: qh * QH + (j + 1) * 512],
                            start=True, stop=True)
                        dst = st[:, j * 512:(j + 1) * 512]
                        if (2 * kc + j) % 3 == 1:
                            scr = stp.tile([P, 512], F32, tag="scr", bufs=4,
                                           name=f"scr{it}_{p}_{kc}_{j}")
                            nc.vector.tensor_copy(scr[:], sc[:])
                            nc.gpsimd.tensor_tensor(
                                out=dst, in0=expbase[:, 0:512], in1=scr[:],
                                op=ALU.pow)
                        else:
                            nc.scalar.activation(dst, sc[:], AF.Exp,
                                                 scale=SCALE)

                def emit_av(pprev, qc, ghost=False, it=it):
                    """One full q-chunk: 16-matmul PSUM group + normalize."""
                    h, qh = pprev // 2, pprev % 2
                    pool, tag = ((psc, "sc") if ghost and qc % 2 else
                                 (psx, "ps1"))
                    ps = pool.tile([P, 512], F32, tag=tag,
                                   name=f"av{it}_{pprev}_{qc}")
                    stt = st_tiles[pprev % 2]
                    for kcc in range(MS):
                        nc.tensor.matmul(ps[:, 0:HD + 1],
                                         stt[kcc][:, qc * P:(qc + 1) * P],
                                         vv[kcc][:, h, :],
                                         start=(kcc == 0), stop=(kcc == MS - 1))
                    qg = qh * 8 + qc
                    rec = recp.tile([P, 1], F32, tag="rec", bufs=4,
                                    name=f"rec{it}_{pprev}_{qc}")
                    nc.vector.reciprocal(rec[:], ps[:, HD:HD + 1])
                    nc.vector.tensor_scalar_mul(
                        out_sb[h][:, qg * HD:(qg + 1) * HD],
                        ps[:, 0:HD], rec[:])
                    if ghost and qc == 5:
                        nc.sync.dma_start(out=outd[7 * P:8 * P, 512:896],
                                          in_=out_sb[7][:, 512:896])
                    elif ghost and qc == 7:
                        nc.sync.dma_start(out=outd[7 * P:8 * P, 896:1024],
                                          in_=out_sb[7][:, 896:1024])

                for p in range(NPASS + 1):
                    for kc in range(MS):
                        if p < NPASS:
                            emit_qk(p, kc)
                        if p >= 1 and kc >= 8:
                            emit_av(p - 1, kc - 8, ghost=(p == NPASS))
                        for spec in sched.pop((p, kc), ()):
                            emit_quantum(spec)
                    if 1 <= p < NPASS:
                        # AV(p-1) = (head (p-1)//2, q-half (p-1)%2) finished
                        h, half = (p - 1) // 2, (p - 1) % 2
                        nc.sync.dma_start(
                            out=outd[h * P:(h + 1) * P, half * 512:(half + 1) * 512],
                            in_=out_sb[h][:, half * 512:(half + 1) * 512])

                assert not sched, f"unemitted quanta: {list(sched)}"
                stp.release()

    nc.finalize()
    return nc


_NC_CACHE = {}


def _get_nc(iters=1):
    if iters not in _NC_CACHE:
        _NC_CACHE[iters] = _build(iters)
    return _NC_CACHE[iters]


def _permute_wqk(w):
    # [k*128+p, m*128+j] -> [m*128+p, k*128+j]: one m-tile contiguous per row
    w4 = w.reshape(KD, P, KD, P)
    return np.ascontiguousarray(w4.transpose(2, 1, 0, 3).reshape(D, D))


def make_in_maps(inputs, W_qkv, b_qkv):
    BF = np.float16
    inputs = np.asarray(inputs, dtype=np.float32)
    W = np.asarray(W_qkv, dtype=np.float32)
    b = np.asarray(b_qkv, dtype=np.float32)
    xt_by_b = [np.ascontiguousarray(inputs[bi].T).astype(BF)
               for bi in range(B_FULL)]
    in_maps = []
    for c in range(N_CORES):
        bi, hg = c // 2, c % 2
        c0 = hg * QKC
        bqk_cat = np.concatenate([b[c0:c0 + QKC], b[D + c0:D + c0 + QKC]])
        in_maps.append({
            "xt": xt_by_b[bi],
            "wqk": _permute_wqk(np.concatenate(
                [W[:, c0:c0 + QKC], W[:, D + c0:D + c0 + QKC]],
                axis=1)).astype(BF),
            "wv": np.ascontiguousarray(W[:, 2 * D + c0:2 * D + c0 + QKC]).astype(BF),
            "bqk": np.ascontiguousarray(bqk_cat.reshape(KD, P).T),
            "bvb": np.ascontiguousarray(np.broadcast_to(
                b[2 * D + c0:2 * D + c0 + QKC], (P, QKC))),
        })
    return in_maps


def assemble(results, B=B_FULL):
    out = np.empty((B, S, D), dtype=np.float32)
    for c in range(N_CORES):
        bi, hg = c // 2, c % 2
        arr = np.asarray(results[c]["outd"]).reshape(NHC, P, MS, HD)
        arr = arr.transpose(2, 1, 0, 3).reshape(S, QKC)
        out[bi, :, hg * QKC:(hg + 1) * QKC] = arr.astype(np.float32)
    return out


def kernel(inputs, mask, W_qkv, b_qkv):
    # mask is all-True for this problem (spec: fill=ones); softmax unaffected.
    nc = _get_nc()
    in_maps = make_in_maps(inputs, W_qkv, b_qkv)
    res = run_bass_kernel_spmd(nc, in_maps, core_ids=list(range(N_CORES)))
    return assemble(res.results)
